# revision 19
# baseline (speedup 1.0000x reference)
"""Trainium2 Bass kernel for nn_MultiHeadAttention_76038101008807.

Causal ALiBi multi-head attention, B=2 S=2048 F=1024 H=16 (head_dim 64).

Sharding: 8 NeuronCores = data parallel over batch (2) x tensor parallel over
heads. Heads are grouped STRIDED (core g of a batch gets heads {g, g+4, g+8,
g+12}) so that every core sees the same multiset of ALiBi band widths -- the
kept-tile structure is identical across cores (SPMD-safe) and balanced.

ALiBi banding: head h attends effectively only to kv in [q - W_h, q] with
W_h = ceil(TH/slope_h); contributions beyond that are < e^-TH relative and
vanish in fp32. Each of the 4 head "slots" uses the max width over the cores'
heads in that slot -> static per-slot band [full, full, ~720, ~180].

Per core: QKV projections from xT (bf16 matmuls, fp32 PSUM), causal+banded
ALiBi attention in a transposed layout (scoresT [kv, q]; exp on ACT with a
per-(slot,kv-tile) fp32 bias; -slope*q rides as a 65th contraction row), P@V
with an appended ones-column on V for the softmax denominators, reciprocal
broadcast via a rank-1 PE matmul into PSUM, and a partial output projection
DMA'd straight from SBUF. Host sums the 4 partials per batch and adds b_out.
"""

import math
from collections import deque
from contextlib import ExitStack

import numpy as np

import concourse.bass as bass
import concourse.bacc as bacc_mod
import concourse.tile as tile
import concourse.mybir as mybir

F32 = mybir.dt.float32
BF16 = mybir.dt.bfloat16

TH = 25.0  # exp(-TH) cutoff for the ALiBi band (dropped mass < e^-13 rel)


def band_tiles(ci, W, CHUNK=512, KT=128):
    """Kept kv-subtiles for q-chunk ci under band width W.

    Returns [(kj, lo, hi, diag, first, last)]; col range [lo, hi) within the
    chunk. First tile is forced full-width so the PSUM accumulation group
    covers every column (start=True writes [0, CHUNK))."""
    nkj = (ci + 1) * (CHUNK // KT)
    kjmin = max(0, -(-(ci * CHUNK - W - (KT - 1)) // KT))
    out = []
    for kj in range(kjmin, nkj):
        joff = kj * KT - ci * CHUNK
        lo = max(0, joff)
        # hi stays CHUNK: partial-width PSUM accumulation miscompiles on HW
        # (race in lowering); the band savings come from skipped tiles + the
        # lo-side diagonal slice, which stays single-MM-group.
        out.append((kj, lo, CHUNK, joff >= 0, kj == kjmin, kj == nkj - 1))
    return out


def build_nc(S=2048, F=1024, HPC=4, CHUNK=512, mm_dt="bf16",
             Wslots=(2048, 2048, 720, 180), SKEW=3):
    D = 64
    KT = 128
    NPAIR = HPC // 2
    NCI = S // CHUNK
    KF = F // 128
    NKT = S // KT
    DT = BF16
    assert mm_dt == "bf16"

    nc = bacc_mod.Bacc("TRN2", target_bir_lowering=False, debug=False)
    xT_d = nc.dram_tensor("xT", [F, S], DT, kind="ExternalInput")
    # packed weights: one DMA each, 4KB contiguous per partition line
    wq_d = nc.dram_tensor("wq", [128, KF * 128 * NPAIR], DT, kind="ExternalInput")
    wk_d = nc.dram_tensor("wk", [128, KF * 128 * NPAIR], DT, kind="ExternalInput")
    wv_d = nc.dram_tensor("wv", [128, KF * 64 * HPC], DT, kind="ExternalInput")
    wout_d = nc.dram_tensor("wout", [128, NPAIR * F], DT, kind="ExternalInput")
    bqk_d = nc.dram_tensor("bqk", [64, 2 * NPAIR * 2], F32, kind="ExternalInput")
    bv_d = nc.dram_tensor("bv", [1, 64 * HPC], DT, kind="ExternalInput")
    aux_d = nc.dram_tensor("aux", [HPC * 2, S], DT, kind="ExternalInput")
    ab_d = nc.dram_tensor("ab", [128, HPC * NKT], F32, kind="ExternalInput")
    y_d = nc.dram_tensor("y", [S, F], F32, kind="ExternalOutput")

    with tile.TileContext(nc) as tc, ExitStack() as ctx:
        persist = ctx.enter_context(tc.tile_pool(name="persist", bufs=1))

        qT = [persist.tile([65, S], DT, tag=f"qT{j}", name=f"qT{j}")
              for j in range(HPC)]
        kT = [persist.tile([65, S], DT, tag=f"kT{j}", name=f"kT{j}")
              for j in range(HPC)]
        v_all = persist.tile([128, NKT, HPC, 65], DT, tag="v_all", name="v_all")
        attnT = [[persist.tile([128, CHUNK], DT, tag=f"attnT{p}_{c}",
                               name=f"attnT{p}_{c}")
                  for p in range(NPAIR)] for c in range(NCI)]
        wout_t = persist.tile([128, NPAIR, F], DT, tag="wout", name="wout")
        bqk_t = persist.tile([64, 2 * NPAIR * 2], F32, tag="bqk", name="bqk")
        bv_t = persist.tile([1, 64 * HPC], DT, tag="bv", name="bv")
        ab_t = persist.tile([128, HPC * NKT], F32, tag="ab", name="ab")
        onesb = persist.tile([1, 128], DT, tag="onesb", name="onesb")
        onesf = persist.tile([1, 64], F32, tag="onesf", name="onesf")

        # ---------------- Phase 1: QKV projections ----------------
        with (
            tc.tile_pool(name="xw", bufs=1) as xw,
            tc.tile_pool(name="qk_ps", bufs=4, space="PSUM") as qk_ps,
            tc.tile_pool(name="v_ps", bufs=2, space="PSUM") as v_ps,
        ):
            xt = [xw.tile([128, S], DT, tag=f"x{k}", name=f"x{k}")
                  for k in range(KF)]
            wq_t = xw.tile([128, NPAIR, KF, 128], DT, tag="wq", name="wq_t")
            wk_t = xw.tile([128, NPAIR, KF, 128], DT, tag="wk", name="wk_t")
            wv_t = xw.tile([128, KF, 64 * HPC], DT, tag="wv", name="wv_t")

            # DMA order: what phase-1 compute needs first, biggest lines.
            # wq/wk pair-halves (contiguous) so the first psum's weights
            # arrive first.
            nc.sync.dma_start(wq_t[:, 0].rearrange("p a b -> p (a b)"),
                              wq_d[:, 0:KF * 128])
            for k in range(KF):
                nc.sync.dma_start(xt[k][:, 0:CHUNK],
                                  xT_d[k * 128:(k + 1) * 128, 0:CHUNK])
            nc.sync.dma_start(wk_t[:, 0].rearrange("p a b -> p (a b)"),
                              wk_d[:, 0:KF * 128])
            nc.sync.dma_start(wq_t[:, 1].rearrange("p a b -> p (a b)"),
                              wq_d[:, KF * 128:])
            nc.sync.dma_start(wk_t[:, 1].rearrange("p a b -> p (a b)"),
                              wk_d[:, KF * 128:])
            nc.sync.dma_start(wv_t[:].rearrange("p a b -> p (a b)"), wv_d[:])
            for j in range(HPC):
                nc.sync.dma_start(qT[j][64:65, :], aux_d[2 * j:2 * j + 1, :])
                nc.sync.dma_start(kT[j][64:65, :], aux_d[2 * j + 1:2 * j + 2, :])
            nc.sync.dma_start(bqk_t[:], bqk_d[:])
            nc.sync.dma_start(bv_t[:], bv_d[:])
            nc.sync.dma_start(ab_t[:], ab_d[:])
            for k in range(KF):
                nc.sync.dma_start(xt[k][:, CHUNK:S],
                                  xT_d[k * 128:(k + 1) * 128, CHUNK:S])
            nc.sync.dma_start(wout_t[:].rearrange("p a b -> p (a b)"), wout_d[:])

            nc.vector.memset(onesb[:], 1.0)
            nc.vector.memset(onesf[:], 1.0)
            nc.vector.memset(v_all[:, :, :, 64:65], 1.0)

            for ci in range(NCI):
                csl = slice(ci * CHUNK, (ci + 1) * CHUNK)
                for p in range(NPAIR):
                    for qk in range(2):
                        w_t = wq_t if qk == 0 else wk_t
                        dst = qT if qk == 0 else kT
                        ps = qk_ps.tile([128, CHUNK], F32, tag="qkps",
                                        name="qkps")
                        for k in range(KF):
                            nc.tensor.matmul(
                                ps[:],
                                w_t[:, p, k, :],
                                xt[k][:, csl],
                                start=(k == 0), stop=(k == KF - 1),
                            )
                        for m in range(2):
                            j = 2 * p + m
                            bcol = (qk * NPAIR + p) * 2 + m
                            # bias-add on DVE (ACT is the attention-phase
                            # bottleneck; DVE is idle in phase 1)
                            nc.vector.tensor_scalar(
                                out=dst[j][0:64, csl],
                                in0=ps[64 * m:64 * m + 64, :],
                                scalar1=bqk_t[:, bcol:bcol + 1],
                                scalar2=None,
                                op0=mybir.AluOpType.add,
                            )
                for st in range(4 * ci, 4 * ci + 4):
                    ps = v_ps.tile([128, 64 * HPC], F32, tag="vps", name="vps")
                    for k in range(KF):
                        nc.tensor.matmul(
                            ps[:],
                            xt[k][:, st * 128:(st + 1) * 128],
                            wv_t[:, k, :],
                            start=(k == 0), stop=False,
                        )
                    nc.tensor.matmul(
                        ps[:], onesb[:, 0:128], bv_t[:],
                        start=False, stop=True,
                    )
                    p_ap = ps[:]
                    ps3 = bass.AP(
                        tensor=p_ap.tensor, offset=p_ap.offset,
                        ap=[list(p_ap.ap[0]), [64, HPC], [1, 64]])
                    nc.vector.tensor_copy(v_all[:, st, :, 0:64], ps3)

        # ---------------- Phase 2: banded attention + out projection --------
        with (
            tc.tile_pool(name="sc_ps", bufs=3, space="PSUM") as sc_ps,
            tc.tile_pool(name="at_ps", bufs=2, space="PSUM") as at_ps,
            tc.tile_pool(name="bc_ps", bufs=1, space="PSUM") as bc_ps,
            tc.tile_pool(name="out_ps", bufs=2, space="PSUM") as out_ps,
            tc.tile_pool(name="pt", bufs=6) as pt_pool,
            tc.tile_pool(name="sm", bufs=8) as sm_pool,
            tc.tile_pool(name="outsb", bufs=2) as out_pool,
        ):
            FOC = 512
            NFO = F // FOC

            def outproj_groups(ci):
                """Closures for chunk ci's out-projection, one per (qt, fo)."""
                groups = []
                box = {}
                for qt in range(CHUNK // 128):
                    for fo in range(NFO):
                        def g(ci=ci, qt=qt, fo=fo):
                            q0 = ci * CHUNK + qt * 128
                            if fo == 0:
                                box[qt] = out_pool.tile([128, F], F32,
                                                        tag="osb", name="osbt")
                            osb = box[qt]
                            op = out_ps.tile([128, FOC], F32, tag="op",
                                             name="opps")
                            for p in range(NPAIR):
                                nc.tensor.matmul(
                                    op[:],
                                    attnT[ci][p][:, qt * 128:(qt + 1) * 128],
                                    wout_t[:, p, fo * FOC:(fo + 1) * FOC],
                                    start=(p == 0), stop=(p == NPAIR - 1),
                                )
                            nc.vector.tensor_copy(
                                osb[:, fo * FOC:(fo + 1) * FOC], op[:])
                            if fo == NFO - 1:
                                nc.sync.dma_start(y_d[q0:q0 + 128, :], osb[:])
                        groups.append(g)
                return groups

            def run_chunk(ci, op_groups):
                tiles = []
                for j in range(HPC):
                    for t in band_tiles(ci, Wslots[j], CHUNK, KT):
                        tiles.append((j,) + t)
                pend = deque()
                deferred = []  # (countdown, closure) in PE-stream ticks
                at_cur = {}
                opq = deque(op_groups)
                op_every = max(2, len(tiles) // len(opq)) if opq else 0
                nticks = 0

                def tick():
                    nonlocal nticks
                    nticks += 1
                    for d in list(deferred):
                        d[0] -= 1
                        if d[0] <= 0:
                            deferred.remove(d)
                            d[1]()
                    if opq and op_every and nticks % op_every == 0:
                        opq.popleft()()

                def emit_attnv(j, kj, lo, hi, first, last, pt):
                    if first:
                        at_cur[j] = at_ps.tile([65, CHUNK], F32, tag="at",
                                               name="atps")
                    at = at_cur[j]
                    nc.tensor.matmul(
                        at[:, lo:hi],
                        v_all[:, kj, j, :],
                        pt[:, lo:hi],
                        start=first, stop=last,
                    )
                    if last:
                        stg = sm_pool.tile([64, CHUNK], F32, tag="stg",
                                           name="stgt")
                        nc.vector.tensor_copy(stg[:], at[0:64, :])
                        den = sm_pool.tile([1, CHUNK], F32, tag="den",
                                           name="dent")
                        nc.vector.tensor_copy(den[:], at[64:65, :])
                        rcp = sm_pool.tile([1, CHUNK], F32, tag="rcp",
                                           name="rcpt")
                        nc.vector.reciprocal_approx_fast(rcp[:], den[:])

                        def tail(jj=j, stg=stg, rcp=rcp):
                            bcps = bc_ps.tile([64, CHUNK], F32, tag="bc",
                                              name="bcps")
                            nc.tensor.matmul(bcps[:], onesf[:],
                                             rcp[:], start=True, stop=True)
                            p, m = divmod(jj, 2)
                            nc.vector.tensor_tensor(
                                out=attnT[ci][p][64 * m:64 * m + 64, :],
                                in0=stg[0:64, :], in1=bcps[:],
                                op=mybir.AluOpType.mult)
                        deferred.append([3, tail])

                for (j, kj, lo, hi, diag, first, last) in tiles:
                    sp = sc_ps.tile([128, CHUNK], F32, tag="sc", name="scps")
                    joff = kj * KT - ci * CHUNK
                    nc.tensor.matmul(
                        sp[:, lo:hi],
                        kT[j][0:65, kj * KT:(kj + 1) * KT],
                        qT[j][0:65, ci * CHUNK + lo:ci * CHUNK + hi],
                        start=True, stop=True,
                    )
                    tick()
                    pt = pt_pool.tile([128, CHUNK], DT, tag="pt", name="ptt")
                    nc.scalar.activation(
                        pt[:, lo:hi], sp[:, lo:hi],
                        mybir.ActivationFunctionType.Exp,
                        bias=ab_t[:, j * NKT + kj:j * NKT + kj + 1])
                    if diag:
                        # full width: fills the unwritten [0, lo) margin with
                        # zeros (predicate is false there), so attnV can read
                        # the whole tile
                        nc.gpsimd.affine_select(
                            pt[:], pt[:],
                            pattern=[[1, CHUNK]],
                            base=-joff,
                            channel_multiplier=-1,
                            compare_op=mybir.AluOpType.is_ge,
                            fill=0.0,
                        )
                    pend.append((j, kj, 0, CHUNK, first, last, pt))
                    if len(pend) > SKEW:
                        emit_attnv(*pend.popleft())
                        tick()
                while pend:
                    emit_attnv(*pend.popleft())
                    tick()
                while deferred:
                    d = deferred.pop(0)
                    d[1]()
                while opq:
                    opq.popleft()()

            for ci in range(NCI):
                run_chunk(ci, outproj_groups(ci - 1) if ci > 0 else [])
            for g in outproj_groups(NCI - 1):
                g()

    nc.finalize()
    return nc


def make_host_inputs(x, W_qkv, b_qkv, W_out, slopes, core, HPC=4,
                     mm_dt="bf16", S=None, F=None):
    """Build the per-core input map (numpy) from full problem inputs."""
    import ml_dtypes
    B, S_, F_ = x.shape
    S = S or S_
    F = F or F_
    D = 64
    KT = 128
    KF = F // 128
    NKT = S // KT
    H = W_qkv.shape[1] // 3 // D
    NPAIR = HPC // 2
    n_hg = H // HPC
    b = core // n_hg
    g = core % n_hg
    # strided head grouping, slot order widest band -> narrowest
    heads = [4 * (HPC - 1 - j) + g for j in range(HPC)]
    np_dt = ml_dtypes.bfloat16 if mm_dt == "bf16" else np.float32

    W = W_qkv.reshape(F, 3, H, D)
    bq = b_qkv.reshape(3, H, D)
    scale = 1.0 / np.sqrt(D)

    xT = np.ascontiguousarray(x[b].T)

    def pack(wcat):  # [F, C] -> [128, KF*C]
        C = wcat.shape[1]
        return np.ascontiguousarray(
            wcat.reshape(KF, 128, C).transpose(1, 0, 2).reshape(128, KF * C))

    def pack_pk(wcat):  # [F, 256] -> [128, NPAIR*KF*128], pair-major
        w = wcat.reshape(KF, 128, NPAIR, 128)
        return np.ascontiguousarray(w.transpose(1, 2, 0, 3).reshape(128, -1))

    wq = pack_pk(np.concatenate([W[:, 0, h, :] for h in heads], axis=1) * scale)
    wk = pack_pk(np.concatenate([W[:, 1, h, :] for h in heads], axis=1))
    wv = pack(np.concatenate([W[:, 2, h, :] for h in heads], axis=1))
    wout_cat = np.concatenate([W_out[h * D:(h + 1) * D, :] for h in heads],
                              axis=0)  # [HPC*64, F]
    wout = np.ascontiguousarray(
        wout_cat.reshape(NPAIR, 128, F).transpose(1, 0, 2).reshape(128, -1))

    bqk = np.zeros((64, 2 * NPAIR * 2), np.float32)
    for p in range(NPAIR):
        for m in range(2):
            h = heads[2 * p + m]
            bqk[:, (0 * NPAIR + p) * 2 + m] = bq[0, h] * scale
            bqk[:, (1 * NPAIR + p) * 2 + m] = bq[1, h]
    bv = np.concatenate([bq[2, h] for h in heads])[None, :]

    aux = np.zeros((HPC * 2, S), np.float32)
    idx = np.arange(S, dtype=np.float32)
    for j, h in enumerate(heads):
        sl = float(slopes[h])
        aux[2 * j + 0] = -sl * idx
        aux[2 * j + 1] = 1.0

    ab = np.zeros((128, HPC * NKT), np.float32)
    kvp = np.arange(128, dtype=np.float32)
    for j, h in enumerate(heads):
        sl = float(slopes[h])
        for kj in range(NKT):
            ab[:, j * NKT + kj] = sl * (kj * KT + kvp)
    return {
        "xT": xT.astype(np_dt), "wq": wq.astype(np_dt), "wk": wk.astype(np_dt),
        "wv": wv.astype(np_dt), "wout": wout.astype(np_dt),
        "bqk": bqk, "bv": bv.astype(np_dt), "aux": aux.astype(np_dt), "ab": ab,
    }


def combine_outputs(results, b_out, B, n_hg):
    """Sum partial y's per batch, add bias."""
    S, F = results[0]["y"].shape
    y = np.zeros((B, S, F), np.float32)
    for core, r in enumerate(results):
        y[core // n_hg] += r["y"]
    return y + np.asarray(b_out, np.float32)[None, None, :]


def compute_wslots(slopes, S, HPC=4, n_hg=4):
    Ws = []
    for j in range(HPC):
        mx = 0
        for g in range(n_hg):
            h = n_hg * (HPC - 1 - j) + g
            mx = max(mx, int(math.ceil(TH / float(slopes[h]))))
        Ws.append(min(S, mx))
    return tuple(Ws)


_CACHED = {}


def kernel(x, W_qkv, b_qkv, W_out, b_out, slopes):
    """Full inputs in, full output out; shards across 8 NeuronCores inside."""
    from concourse.bass_utils import run_bass_kernel_spmd

    x = np.asarray(x)
    W_qkv = np.asarray(W_qkv)
    b_qkv = np.asarray(b_qkv)
    W_out = np.asarray(W_out)
    b_out = np.asarray(b_out)
    slopes = np.asarray(slopes)

    B, S, F = x.shape          # 2, 2048, 1024
    H = 16
    HPC = 4
    n_hg = H // HPC            # 4 head groups
    n_cores = B * n_hg         # 8

    Ws = compute_wslots(slopes, S, HPC, n_hg)
    key = ("nc", S, F, Ws)
    if key not in _CACHED:
        _CACHED[key] = build_nc(S=S, F=F, HPC=HPC, mm_dt="bf16", Wslots=Ws)
    nc = _CACHED[key]

    in_maps = [
        make_host_inputs(x, W_qkv, b_qkv, W_out, slopes, c, HPC=HPC,
                         mm_dt="bf16")
        for c in range(n_cores)
    ]
    res = run_bass_kernel_spmd(nc, in_maps, list(range(n_cores)))
    return combine_outputs(res.results, b_out, B, n_hg)


# revision 20
# speedup vs baseline: 1.0391x; 1.0391x over previous
"""Trainium2 Bass kernel for nn_MultiHeadAttention_76038101008807.

Causal ALiBi multi-head attention, B=2 S=2048 F=1024 H=16 (head_dim 64).

Sharding: 8 NeuronCores = data parallel over batch (2) x tensor parallel over
heads. Heads are grouped STRIDED (core g of a batch gets heads {g, g+4, g+8,
g+12}) so that every core sees the same multiset of ALiBi band widths -- the
kept-tile structure is identical across cores (SPMD-safe) and balanced.

ALiBi banding: head h attends effectively only to kv in [q - W_h, q] with
W_h = ceil(TH/slope_h); contributions beyond that are < e^-TH relative and
vanish in fp32. Each of the 4 head "slots" uses the max width over the cores'
heads in that slot -> static per-slot band [full, full, ~720, ~180].

Per core: QKV projections from xT (bf16 matmuls, fp32 PSUM), causal+banded
ALiBi attention in a transposed layout (scoresT [kv, q]; exp on ACT with a
per-(slot,kv-tile) fp32 bias; -slope*q rides as a 65th contraction row), P@V
with an appended ones-column on V for the softmax denominators, reciprocal
broadcast via a rank-1 PE matmul into PSUM, and a partial output projection
DMA'd straight from SBUF. Host sums the 4 partials per batch and adds b_out.
"""

import math
from collections import deque
from contextlib import ExitStack

import numpy as np

import concourse.bass as bass
import concourse.bacc as bacc_mod
import concourse.tile as tile
import concourse.mybir as mybir

F32 = mybir.dt.float32
BF16 = mybir.dt.bfloat16

TH = 25.0  # exp(-TH) cutoff for the ALiBi band (dropped mass < e^-13 rel)


def band_tiles(ci, W, CHUNK=512, KT=128):
    """Kept kv-subtiles for q-chunk ci under band width W.

    Returns [(kj, lo, hi, diag, first, last)]; col range [lo, hi) within the
    chunk. First tile is forced full-width so the PSUM accumulation group
    covers every column (start=True writes [0, CHUNK))."""
    nkj = (ci + 1) * (CHUNK // KT)
    kjmin = max(0, -(-(ci * CHUNK - W - (KT - 1)) // KT))
    out = []
    for kj in range(kjmin, nkj):
        joff = kj * KT - ci * CHUNK
        lo = max(0, joff)
        # hi stays CHUNK: partial-width PSUM accumulation miscompiles on HW
        # (race in lowering); the band savings come from skipped tiles + the
        # lo-side diagonal slice, which stays single-MM-group.
        out.append((kj, lo, CHUNK, joff >= 0, kj == kjmin, kj == nkj - 1))
    return out


def build_nc(S=2048, F=1024, HPC=4, CHUNK=512, mm_dt="bf16",
             Wslots=(2048, 2048, 720, 180), SKEW=3):
    D = 64
    KT = 128
    NPAIR = HPC // 2
    NCI = S // CHUNK
    KF = F // 128
    NKT = S // KT
    DT = BF16
    assert mm_dt == "bf16"

    nc = bacc_mod.Bacc("TRN2", target_bir_lowering=False, debug=False)
    xT_d = nc.dram_tensor("xT", [F, S], DT, kind="ExternalInput")
    # packed weights: one DMA each, 4KB contiguous per partition line
    wq_d = nc.dram_tensor("wq", [128, KF * 128 * NPAIR], DT, kind="ExternalInput")
    wk_d = nc.dram_tensor("wk", [128, KF * 128 * NPAIR], DT, kind="ExternalInput")
    wv_d = nc.dram_tensor("wv", [128, KF * 64 * HPC], DT, kind="ExternalInput")
    wout_d = nc.dram_tensor("wout", [128, NPAIR * F], DT, kind="ExternalInput")
    bqk_d = nc.dram_tensor("bqk", [64, 2 * NPAIR * 2], F32, kind="ExternalInput")
    bv_d = nc.dram_tensor("bv", [1, 64 * HPC], DT, kind="ExternalInput")
    aux_d = nc.dram_tensor("aux", [HPC * 2, S], DT, kind="ExternalInput")
    ab_d = nc.dram_tensor("ab", [128, HPC * NKT], F32, kind="ExternalInput")
    y_d = nc.dram_tensor("y", [S, F], F32, kind="ExternalOutput")

    with tile.TileContext(nc) as tc, ExitStack() as ctx:
        persist = ctx.enter_context(tc.tile_pool(name="persist", bufs=1))

        qT = [persist.tile([65, S], DT, tag=f"qT{j}", name=f"qT{j}")
              for j in range(HPC)]
        kT = [persist.tile([65, S], DT, tag=f"kT{j}", name=f"kT{j}")
              for j in range(HPC)]
        v_all = persist.tile([128, NKT, HPC, 65], DT, tag="v_all", name="v_all")
        attnT = [[persist.tile([128, CHUNK], DT, tag=f"attnT{p}_{c}",
                               name=f"attnT{p}_{c}")
                  for p in range(NPAIR)] for c in range(NCI)]
        wout_t = persist.tile([128, NPAIR, F], DT, tag="wout", name="wout")
        bqk_t = persist.tile([64, 2 * NPAIR * 2], F32, tag="bqk", name="bqk")
        bv_t = persist.tile([1, 64 * HPC], DT, tag="bv", name="bv")
        ab_t = persist.tile([128, HPC * NKT], F32, tag="ab", name="ab")
        onesb = persist.tile([1, 128], DT, tag="onesb", name="onesb")
        onesf = persist.tile([1, 64], F32, tag="onesf", name="onesf")

        # ---------------- Phase 1: QKV projections ----------------
        with (
            tc.tile_pool(name="xw", bufs=1) as xw,
            tc.tile_pool(name="qk_ps", bufs=4, space="PSUM") as qk_ps,
            tc.tile_pool(name="v_ps", bufs=2, space="PSUM") as v_ps,
        ):
            xt = [xw.tile([128, S], DT, tag=f"x{k}", name=f"x{k}")
                  for k in range(KF)]
            wq_t = xw.tile([128, NPAIR, KF, 128], DT, tag="wq", name="wq_t")
            wk_t = xw.tile([128, NPAIR, KF, 128], DT, tag="wk", name="wk_t")
            wv_t = xw.tile([128, KF, 64 * HPC], DT, tag="wv", name="wv_t")

            # DMA order: what phase-1 compute needs first, biggest lines
            nc.sync.dma_start(wq_t[:].rearrange("p a b c -> p (a b c)"),
                              wq_d[:])
            for k in range(KF):
                nc.sync.dma_start(xt[k][:, 0:CHUNK],
                                  xT_d[k * 128:(k + 1) * 128, 0:CHUNK])
            nc.sync.dma_start(wk_t[:].rearrange("p a b c -> p (a b c)"),
                              wk_d[:])
            nc.sync.dma_start(wv_t[:].rearrange("p a b -> p (a b)"), wv_d[:])
            for j in range(HPC):
                nc.sync.dma_start(qT[j][64:65, :], aux_d[2 * j:2 * j + 1, :])
                nc.sync.dma_start(kT[j][64:65, :], aux_d[2 * j + 1:2 * j + 2, :])
            nc.sync.dma_start(bqk_t[:], bqk_d[:])
            nc.sync.dma_start(bv_t[:], bv_d[:])
            nc.sync.dma_start(ab_t[:], ab_d[:])
            for k in range(KF):
                nc.sync.dma_start(xt[k][:, CHUNK:S],
                                  xT_d[k * 128:(k + 1) * 128, CHUNK:S])
            nc.sync.dma_start(wout_t[:].rearrange("p a b -> p (a b)"), wout_d[:])

            nc.vector.memset(onesb[:], 1.0)
            nc.vector.memset(onesf[:], 1.0)
            nc.vector.memset(v_all[:, :, :, 64:65], 1.0)

            for ci in range(NCI):
                csl = slice(ci * CHUNK, (ci + 1) * CHUNK)
                for p in range(NPAIR):
                    for qk in range(2):
                        w_t = wq_t if qk == 0 else wk_t
                        dst = qT if qk == 0 else kT
                        ps = qk_ps.tile([128, CHUNK], F32, tag="qkps",
                                        name="qkps")
                        for k in range(KF):
                            nc.tensor.matmul(
                                ps[:],
                                w_t[:, p, k, :],
                                xt[k][:, csl],
                                start=(k == 0), stop=(k == KF - 1),
                            )
                        for m in range(2):
                            j = 2 * p + m
                            bcol = (qk * NPAIR + p) * 2 + m
                            # bias-add on DVE (ACT is the attention-phase
                            # bottleneck; DVE is idle in phase 1)
                            nc.vector.tensor_scalar(
                                out=dst[j][0:64, csl],
                                in0=ps[64 * m:64 * m + 64, :],
                                scalar1=bqk_t[:, bcol:bcol + 1],
                                scalar2=None,
                                op0=mybir.AluOpType.add,
                            )
                for st in range(4 * ci, 4 * ci + 4):
                    ps = v_ps.tile([128, 64 * HPC], F32, tag="vps", name="vps")
                    for k in range(KF):
                        nc.tensor.matmul(
                            ps[:],
                            xt[k][:, st * 128:(st + 1) * 128],
                            wv_t[:, k, :],
                            start=(k == 0), stop=False,
                        )
                    nc.tensor.matmul(
                        ps[:], onesb[:, 0:128], bv_t[:],
                        start=False, stop=True,
                    )
                    p_ap = ps[:]
                    ps3 = bass.AP(
                        tensor=p_ap.tensor, offset=p_ap.offset,
                        ap=[list(p_ap.ap[0]), [64, HPC], [1, 64]])
                    nc.vector.tensor_copy(v_all[:, st, :, 0:64], ps3)

        # ---------------- Phase 2: banded attention + out projection --------
        with (
            tc.tile_pool(name="sc_ps", bufs=3, space="PSUM") as sc_ps,
            tc.tile_pool(name="at_ps", bufs=2, space="PSUM") as at_ps,
            tc.tile_pool(name="bc_ps", bufs=1, space="PSUM") as bc_ps,
            tc.tile_pool(name="out_ps", bufs=2, space="PSUM") as out_ps,
            tc.tile_pool(name="pt", bufs=6) as pt_pool,
            tc.tile_pool(name="sm", bufs=8) as sm_pool,
            tc.tile_pool(name="outsb", bufs=2) as out_pool,
        ):
            FOC = 512
            NFO = F // FOC

            def outproj_groups(ci):
                """Closures for chunk ci's out-projection, one per (qt, fo)."""
                groups = []
                box = {}
                for qt in range(CHUNK // 128):
                    for fo in range(NFO):
                        def g(ci=ci, qt=qt, fo=fo):
                            q0 = ci * CHUNK + qt * 128
                            if fo == 0:
                                box[qt] = out_pool.tile([128, F], F32,
                                                        tag="osb", name="osbt")
                            osb = box[qt]
                            op = out_ps.tile([128, FOC], F32, tag="op",
                                             name="opps")
                            for p in range(NPAIR):
                                nc.tensor.matmul(
                                    op[:],
                                    attnT[ci][p][:, qt * 128:(qt + 1) * 128],
                                    wout_t[:, p, fo * FOC:(fo + 1) * FOC],
                                    start=(p == 0), stop=(p == NPAIR - 1),
                                )
                            nc.vector.tensor_copy(
                                osb[:, fo * FOC:(fo + 1) * FOC], op[:])
                            if fo == NFO - 1:
                                nc.sync.dma_start(y_d[q0:q0 + 128, :], osb[:])
                        groups.append(g)
                return groups

            def run_chunk(ci, op_groups):
                tiles = []
                for j in range(HPC):
                    for t in band_tiles(ci, Wslots[j], CHUNK, KT):
                        tiles.append((j,) + t)
                pend = deque()
                deferred = []  # (countdown, closure) in PE-stream ticks
                at_cur = {}
                opq = deque(op_groups)
                op_every = max(2, len(tiles) // len(opq)) if opq else 0
                nticks = 0

                def tick():
                    nonlocal nticks
                    nticks += 1
                    for d in list(deferred):
                        d[0] -= 1
                        if d[0] <= 0:
                            deferred.remove(d)
                            d[1]()
                    if opq and op_every and nticks % op_every == 0:
                        opq.popleft()()

                def emit_attnv(j, kj, lo, hi, first, last, pt):
                    if first:
                        at_cur[j] = at_ps.tile([65, CHUNK], F32, tag="at",
                                               name="atps")
                    at = at_cur[j]
                    nc.tensor.matmul(
                        at[:, lo:hi],
                        v_all[:, kj, j, :],
                        pt[:, lo:hi],
                        start=first, stop=last,
                    )
                    if last:
                        stg = sm_pool.tile([64, CHUNK], F32, tag="stg",
                                           name="stgt")
                        nc.vector.tensor_copy(stg[:], at[0:64, :])
                        den = sm_pool.tile([1, CHUNK], F32, tag="den",
                                           name="dent")
                        nc.vector.tensor_copy(den[:], at[64:65, :])
                        rcp = sm_pool.tile([1, CHUNK], F32, tag="rcp",
                                           name="rcpt")
                        nc.vector.reciprocal_approx_fast(rcp[:], den[:])

                        def tail(jj=j, stg=stg, rcp=rcp):
                            bcps = bc_ps.tile([64, CHUNK], F32, tag="bc",
                                              name="bcps")
                            nc.tensor.matmul(bcps[:], onesf[:],
                                             rcp[:], start=True, stop=True)
                            p, m = divmod(jj, 2)
                            nc.vector.tensor_tensor(
                                out=attnT[ci][p][64 * m:64 * m + 64, :],
                                in0=stg[0:64, :], in1=bcps[:],
                                op=mybir.AluOpType.mult)
                        deferred.append([3, tail])

                for (j, kj, lo, hi, diag, first, last) in tiles:
                    sp = sc_ps.tile([128, CHUNK], F32, tag="sc", name="scps")
                    joff = kj * KT - ci * CHUNK
                    nc.tensor.matmul(
                        sp[:, lo:hi],
                        kT[j][0:65, kj * KT:(kj + 1) * KT],
                        qT[j][0:65, ci * CHUNK + lo:ci * CHUNK + hi],
                        start=True, stop=True,
                    )
                    tick()
                    pt = pt_pool.tile([128, CHUNK], DT, tag="pt", name="ptt")
                    nc.scalar.activation(
                        pt[:, lo:hi], sp[:, lo:hi],
                        mybir.ActivationFunctionType.Exp,
                        bias=ab_t[:, j * NKT + kj:j * NKT + kj + 1])
                    if diag:
                        # full width: fills the unwritten [0, lo) margin with
                        # zeros (predicate is false there), so attnV can read
                        # the whole tile
                        nc.gpsimd.affine_select(
                            pt[:], pt[:],
                            pattern=[[1, CHUNK]],
                            base=-joff,
                            channel_multiplier=-1,
                            compare_op=mybir.AluOpType.is_ge,
                            fill=0.0,
                        )
                    pend.append((j, kj, 0, CHUNK, first, last, pt))
                    if len(pend) > SKEW:
                        emit_attnv(*pend.popleft())
                        tick()
                while pend:
                    emit_attnv(*pend.popleft())
                    tick()
                while deferred:
                    d = deferred.pop(0)
                    d[1]()
                while opq:
                    opq.popleft()()

            for ci in range(NCI):
                run_chunk(ci, outproj_groups(ci - 1) if ci > 0 else [])
            for g in outproj_groups(NCI - 1):
                g()

    nc.finalize()
    return nc


def make_host_inputs(x, W_qkv, b_qkv, W_out, slopes, core, HPC=4,
                     mm_dt="bf16", S=None, F=None):
    """Build the per-core input map (numpy) from full problem inputs."""
    import ml_dtypes
    B, S_, F_ = x.shape
    S = S or S_
    F = F or F_
    D = 64
    KT = 128
    KF = F // 128
    NKT = S // KT
    H = W_qkv.shape[1] // 3 // D
    NPAIR = HPC // 2
    n_hg = H // HPC
    b = core // n_hg
    g = core % n_hg
    # strided head grouping, slot order widest band -> narrowest
    heads = [4 * (HPC - 1 - j) + g for j in range(HPC)]
    np_dt = ml_dtypes.bfloat16 if mm_dt == "bf16" else np.float32

    W = W_qkv.reshape(F, 3, H, D)
    bq = b_qkv.reshape(3, H, D)
    scale = 1.0 / np.sqrt(D)

    xT = np.ascontiguousarray(x[b].T)

    def pack(wcat):  # [F, C] -> [128, KF*C]
        C = wcat.shape[1]
        return np.ascontiguousarray(
            wcat.reshape(KF, 128, C).transpose(1, 0, 2).reshape(128, KF * C))

    def pack_pk(wcat):  # [F, 256] -> [128, NPAIR*KF*128], pair-major
        w = wcat.reshape(KF, 128, NPAIR, 128)
        return np.ascontiguousarray(w.transpose(1, 2, 0, 3).reshape(128, -1))

    wq = pack_pk(np.concatenate([W[:, 0, h, :] for h in heads], axis=1) * scale)
    wk = pack_pk(np.concatenate([W[:, 1, h, :] for h in heads], axis=1))
    wv = pack(np.concatenate([W[:, 2, h, :] for h in heads], axis=1))
    wout_cat = np.concatenate([W_out[h * D:(h + 1) * D, :] for h in heads],
                              axis=0)  # [HPC*64, F]
    wout = np.ascontiguousarray(
        wout_cat.reshape(NPAIR, 128, F).transpose(1, 0, 2).reshape(128, -1))

    bqk = np.zeros((64, 2 * NPAIR * 2), np.float32)
    for p in range(NPAIR):
        for m in range(2):
            h = heads[2 * p + m]
            bqk[:, (0 * NPAIR + p) * 2 + m] = bq[0, h] * scale
            bqk[:, (1 * NPAIR + p) * 2 + m] = bq[1, h]
    bv = np.concatenate([bq[2, h] for h in heads])[None, :]

    aux = np.zeros((HPC * 2, S), np.float32)
    idx = np.arange(S, dtype=np.float32)
    for j, h in enumerate(heads):
        sl = float(slopes[h])
        aux[2 * j + 0] = -sl * idx
        aux[2 * j + 1] = 1.0

    ab = np.zeros((128, HPC * NKT), np.float32)
    kvp = np.arange(128, dtype=np.float32)
    for j, h in enumerate(heads):
        sl = float(slopes[h])
        for kj in range(NKT):
            ab[:, j * NKT + kj] = sl * (kj * KT + kvp)
    return {
        "xT": xT.astype(np_dt), "wq": wq.astype(np_dt), "wk": wk.astype(np_dt),
        "wv": wv.astype(np_dt), "wout": wout.astype(np_dt),
        "bqk": bqk, "bv": bv.astype(np_dt), "aux": aux.astype(np_dt), "ab": ab,
    }


def combine_outputs(results, b_out, B, n_hg):
    """Sum partial y's per batch, add bias."""
    S, F = results[0]["y"].shape
    y = np.zeros((B, S, F), np.float32)
    for core, r in enumerate(results):
        y[core // n_hg] += r["y"]
    return y + np.asarray(b_out, np.float32)[None, None, :]


def compute_wslots(slopes, S, HPC=4, n_hg=4):
    Ws = []
    for j in range(HPC):
        mx = 0
        for g in range(n_hg):
            h = n_hg * (HPC - 1 - j) + g
            mx = max(mx, int(math.ceil(TH / float(slopes[h]))))
        Ws.append(min(S, mx))
    return tuple(Ws)


_CACHED = {}


def kernel(x, W_qkv, b_qkv, W_out, b_out, slopes):
    """Full inputs in, full output out; shards across 8 NeuronCores inside."""
    from concourse.bass_utils import run_bass_kernel_spmd

    x = np.asarray(x)
    W_qkv = np.asarray(W_qkv)
    b_qkv = np.asarray(b_qkv)
    W_out = np.asarray(W_out)
    b_out = np.asarray(b_out)
    slopes = np.asarray(slopes)

    B, S, F = x.shape          # 2, 2048, 1024
    H = 16
    HPC = 4
    n_hg = H // HPC            # 4 head groups
    n_cores = B * n_hg         # 8

    Ws = compute_wslots(slopes, S, HPC, n_hg)
    key = ("nc", S, F, Ws)
    if key not in _CACHED:
        _CACHED[key] = build_nc(S=S, F=F, HPC=HPC, mm_dt="bf16", Wslots=Ws)
    nc = _CACHED[key]

    in_maps = [
        make_host_inputs(x, W_qkv, b_qkv, W_out, slopes, c, HPC=HPC,
                         mm_dt="bf16")
        for c in range(n_cores)
    ]
    res = run_bass_kernel_spmd(nc, in_maps, list(range(n_cores)))
    return combine_outputs(res.results, b_out, B, n_hg)
